# revision 21
# baseline (speedup 1.0000x reference)
"""COIL-style retrieval scoring kernel for Trainium2 (8 NeuronCores, SPMD).

Problem: nn_BertForSemanticEmbedding_16973710754315
  out[q, n] = sum_{i>=1} mask[q,i] * max_j( where(qid[q,i]==did[n,j], qry[q,i]·doc[n,j], 0) )

Algorithm (docs sharded 16/core, queries replicated), "bucketed COIL":

  * Host partitions the 1000 vocab ids into B=18 buckets (greedy vector
    bin-packing + local repair) such that
      - each bucket holds <=128 query tokens  (matmul stationary M)
      - each (doc, bucket) token count <= 9   (so G=10 with a zero pad slot)
    Tokens can only exact-match within their id's bucket, so each device
    scores 18 bucket-local matmuls [K=64, M=128] x [K=64, N=160] instead of
    a dense 2048x2048 sweep -- ~11x less post-matmul reduce volume.
  * Exact-match discrimination INSIDE the matmul: each token's 63-dim
    augmented vector is [reps(32) | code(id)(30) | bias(1)] with codes +-4
    and bias q:-30 / d:16 (both e4m3-exact; product -480 = code self-dot).
    Matching ids contribute code.code - 480 = 0 exactly; in-bucket
    mismatches contribute <= 416 + |S| - 480 < 0 (the host verifies the
    in-bucket code-gram max <= 416, reseeding codes if needed).  Doc-side
    pad columns are all-zero, so every segment contains an exact 0 => the
    segmented max IS relu(max over matching S): no bias/relu op needed.
  * K=64 lets two buckets run CONCURRENTLY in the PE array via row tiling:
    even bucket in array rows 0-63, odd in rows 64-127 (tile_position is
    auto-derived from the operands' base_partition).  9 slot pairs.
  * dT ships as fp8e4m3 (codes/bias exact, reps ~6e-3 extra rel err;
    mixed bf16 x fp8 matmul works); qw/w2 stay bf16 -- fp8's narrow
    per-partition DMA lines measured slower despite fewer bytes.
  * Inputs are chunked per slot-group on two HWDGE queues (qw on the
    Activation queue, dT + w2 on SP), issued before the tile pools so
    descriptor generation overlaps the framework preamble; the matmul
    stream starts as soon as chunk 0 lands.
  * PSUM: one 2-bank tile per group holds 3 even + 3 odd buckets.  ONE
    DVE segmented reduce_max per group reads both banks straight from
    PSUM ([128, 2, 48, 10] -> bf16 A); ScalarE extract + tensor_max
    trees measured worse (per-op overhead + serial latency).
  * Finale: per bucket ONE accumulating matmul with the mask-scatter
    matrix W (qtok -> query, zero for [CLS]/pads) as stationary and the
    reduced A slice as moving operand; all 18 accumulate into one [16,16]
    PSUM tile.  VectorE copies it out (no ScalarE anywhere => no ACT
    table load blocking the DGE queue); DMA [16,16] f32 per core.

  Measured: 47071 ns (dense-COIL baseline) -> 18576 ns.  Of the total,
  ~13 us is fixed framework preamble + semaphore-sweep epilogue that any
  kernel on this harness pays; the compute body is ~5.5 us, paced by
  input-DMA arrival and the serial DVE reduce chain.
"""

import sys
import numpy as np

for _p in ("/opt/trn_rl_repo",):
    if _p not in sys.path:
        sys.path.insert(0, _p)

import ml_dtypes

BF16 = ml_dtypes.bfloat16

NQ, LQ = 16, 128
ND, LD = 128, 128
D = 32
VOCAB = 1000
NCORES = 8
DSHARD = ND // NCORES   # 16 docs per core
NQTOK = NQ * LQ         # 2048 query tokens

R = 31                  # code dims
CVAL = 4.0              # code magnitude (exact in bf16)
BIAS = float(R * CVAL * CVAL)  # 496 = code self-dot, cancelled by bias dim
GRAM_MAX = 448.0        # forbid in-bucket cross-grams >= this (=> <= 432)
KAUG = D + R + 1        # 64 = contraction dim; 2 buckets pack in the PE
B = 18                  # id buckets
CAP = 9                 # max doc tokens per (doc, bucket)
G = CAP + 1             # segment size incl >=1 zero pad slot
NG = DSHARD * G         # 160 = matmul N per bucket
NSLOT = B // 2          # 9 row-tiled matmul pairs
GRP = 3                 # slots per PSUM bank group
NGRP = NSLOT // GRP     # 3 slot groups
SEG = GRP * DSHARD      # 48 segments per bank
E4M3 = (ml_dtypes.float8_e4m3fn if hasattr(ml_dtypes, "float8_e4m3fn")
        else ml_dtypes.float8_e4m3)

_NC = None


# ---------------------------------------------------------------- host prep

def _pack_buckets(qc, dc):
    """Greedy vector bin-packing of ids into B buckets + local repair.
    qc: [VOCAB] query-token counts; dc: [VOCAB, ND] doc-token counts.
    Returns assign [VOCAB] with per-bucket qload<=128 and cell<=CAP."""
    QCAP = 128
    for seed in range(16):
        rng = np.random.RandomState(seed)
        noise = rng.rand(VOCAB) * 0.5
        order = np.argsort(-(dc.max(axis=1) * 100 + dc.sum(axis=1) + qc + noise))
        assign = np.full(VOCAB, -1, dtype=np.int64)
        cell = np.zeros((B, ND), dtype=np.int64)
        qload = np.zeros(B, dtype=np.int64)
        for v in order:
            nc_ = cell + dc[v][None, :]
            over = np.maximum(nc_ - CAP, 0).sum(axis=1)
            qbad = (qload + qc[v]) > QCAP
            score = (over * 10000 + qbad * 10**8
                     + cell.sum(axis=1) + qload * 2 + rng.rand(B))
            b = int(np.argmin(score))
            assign[v] = b
            cell[b] += dc[v]
            qload[b] += qc[v]

        def violations():
            return int(np.maximum(cell - CAP, 0).sum()
                       + np.maximum(qload - QCAP, 0).sum())

        vi = violations()
        for _ in range(20000):
            if vi == 0:
                break
            ob, od = np.nonzero(cell > CAP)
            if len(ob) == 0:
                oq = np.nonzero(qload > QCAP)[0]
                b0, d0 = int(oq[rng.randint(len(oq))]), None
            else:
                j = rng.randint(len(ob))
                b0, d0 = int(ob[j]), int(od[j])
            cand = np.nonzero((assign == b0) & ((dc[:, d0] > 0) if d0 is not None
                                                else (qc > 0)))[0]
            if len(cand) == 0:
                continue
            v = int(cand[rng.randint(len(cand))])
            nc_ = cell + dc[v][None, :]
            over_add = (np.maximum(nc_ - CAP, 0).sum(axis=1)
                        - np.maximum(cell - CAP, 0).sum(axis=1))
            q_add = (np.maximum(qload + qc[v] - QCAP, 0)
                     - np.maximum(qload - QCAP, 0))
            over_rem = (np.maximum(cell[b0] - CAP, 0).sum()
                        - np.maximum(cell[b0] - dc[v] - CAP, 0).sum())
            q_rem = (max(qload[b0] - QCAP, 0)
                     - max(qload[b0] - qc[v] - QCAP, 0))
            delta = over_add + q_add - over_rem - q_rem
            delta[b0] = 10**9
            b1 = int(np.argmin(delta + rng.rand(B) * 0.01))
            if delta[b1] < 0 or (delta[b1] == 0 and rng.rand() < 0.3):
                assign[v] = b1
                cell[b0] -= dc[v]
                cell[b1] += dc[v]
                qload[b0] -= qc[v]
                qload[b1] += qc[v]
                vi = violations()
        if vi == 0:
            return assign
    raise RuntimeError("bucket packing failed")


def _make_codes(assign, q_present, d_present):
    """[VOCAB, R] codes +-CVAL whose in-bucket co-occurring cross-grams
    stay < GRAM_MAX (so mismatch scores are strictly negative)."""
    for seed in range(64):
        rng = np.random.RandomState(12345 + seed)
        C = np.where(rng.rand(VOCAB, R) < 0.5, -CVAL, CVAL).astype(np.float32)
        gram = C @ C.T
        bad = False
        for b in range(B):
            ids = np.nonzero(assign == b)[0]
            qi = ids[q_present[ids]]
            di = ids[d_present[ids]]
            if len(qi) == 0 or len(di) == 0:
                continue
            g = gram[np.ix_(qi, di)].copy()
            g[qi[:, None] == di[None, :]] = -1e9
            if g.max() >= GRAM_MAX:
                bad = True
                break
        if not bad:
            return C
    raise RuntimeError("code generation failed")


def _prepare(doc_reps, qry_reps, qry_attention_mask, doc_input_ids,
             qry_input_ids):
    """Returns per-core input maps: bucketed, padded, bf16 device layouts."""
    qry_reps = np.asarray(qry_reps, dtype=np.float32).reshape(NQTOK, D)
    doc_reps = np.asarray(doc_reps, dtype=np.float32).reshape(ND * LD, D)
    mask = np.asarray(qry_attention_mask, dtype=np.float32)
    qids = np.asarray(qry_input_ids).astype(np.int64).reshape(NQTOK)
    dids = np.asarray(doc_input_ids).astype(np.int64).reshape(ND, LD)

    qc = np.bincount(qids, minlength=VOCAB)
    dc = np.zeros((VOCAB, ND), dtype=np.int64)
    for n in range(ND):
        dc[:, n] += np.bincount(dids[n], minlength=VOCAB)

    assign = _pack_buckets(qc, dc)
    C = _make_codes(assign, qc > 0, dc.sum(axis=1) > 0)

    # augmented token vectors [*, 64]
    qaug = np.zeros((NQTOK, KAUG), dtype=np.float32)
    qaug[:, :D] = qry_reps
    qaug[:, D:D + R] = C[qids]
    qaug[:, D + R] = -BIAS
    daug = np.zeros((ND * LD, KAUG), dtype=np.float32)
    daug[:, :D] = doc_reps
    daug[:, D:D + R] = C[dids.reshape(-1)]
    daug[:, D + R] = 1.0

    tok_bucket = assign[qids]
    W = mask.copy()
    W[:, 0] = 0.0                                   # skip [CLS]
    # qT [128, NSLOT*128]: slot s cols; even-bucket dims on partitions 0:64,
    # odd on 64:128.  w2 [128, 2*NSLOT*NQ] flat [parity][slot][query].
    qT = np.zeros((128, NSLOT * 128), dtype=np.float32)
    w2 = np.zeros((128, 2, NSLOT, NQ), dtype=np.float32)
    for b in range(B):
        s, par = divmod(b, 2)
        toks = np.nonzero(tok_bucket == b)[0]
        assert len(toks) <= 128, f"bucket {b} has {len(toks)} query tokens"
        prow = slice(0, KAUG) if par == 0 else slice(64, 64 + KAUG)
        qT[prow, s * 128:s * 128 + len(toks)] = qaug[toks].T
        qq, ii = toks // LQ, toks % LQ
        w2[np.arange(len(toks)), par, s, qq] = W[qq, ii]

    # doc-side per core: dT [128, NSLOT*NG]; slot s cols are
    # [doc0: G slots | ... | doc15: G slots], zero padded.
    d_bucket = assign[dids]
    qw_bf = np.concatenate(
        [qT, w2.reshape(128, 2 * NSLOT * NQ)], axis=1).astype(BF16)
    in_maps = []
    for core in range(NCORES):
        dT = np.zeros((128, NSLOT * NG), dtype=np.float32)
        for nl in range(DSHARD):
            n = core * DSHARD + nl
            for b in range(B):
                s, par = divmod(b, 2)
                js = np.nonzero(d_bucket[n] == b)[0]
                assert len(js) <= CAP, f"doc {n} bucket {b}: {len(js)}"
                col = s * NG + nl * G
                prow = slice(0, KAUG) if par == 0 else slice(64, 64 + KAUG)
                dT[prow, col:col + len(js)] = daug[n * LD + js].T
        in_maps.append({"qw": qw_bf, "dT": dT.astype(E4M3)})
    return in_maps


# ---------------------------------------------------------------- program

def _build_program():
    from concourse import bacc, tile, mybir

    bf = mybir.dt.bfloat16
    f32 = mybir.dt.float32

    nc = bacc.Bacc("TRN2", target_bir_lowering=False, debug=False,
                   num_devices=NCORES)

    fp8 = mybir.dt.float8e4
    qw_d = nc.declare_dram_parameter("qw", [128, NSLOT * 128 + 2 * NSLOT * NQ],
                                     bf, isOutput=False)
    dT_d = nc.declare_dram_parameter("dT", [128, NSLOT * NG], fp8,
                                     isOutput=False)
    out_d = nc.declare_dram_parameter("out", [NQ, DSHARD], f32, isOutput=True)

    # Input tensors live outside the tile pools; their DMAs are the FIRST
    # instructions in the TileContext, ahead of the tile-pool allocation
    # barriers, so descriptor generation starts as soon as the engine
    # queues open.  Inputs are split per slot-group so group-0 matmuls
    # start early; qw chunks ride the Activation DGE queue, dT chunks +
    # w2 the SP queue.
    qwA_t = nc.alloc_sbuf_tensor("qwA", [128, GRP * 128], bf)
    qwB_t = nc.alloc_sbuf_tensor(
        "qwB", [128, 2 * GRP * 128 + 2 * NSLOT * NQ], bf)
    qwA, qwB = qwA_t.ap(), qwB_t.ap()
    QWg = [qwA, qwB[:, 0:GRP * 128], qwB[:, GRP * 128:2 * GRP * 128]]
    W2 = qwB[:, 2 * GRP * 128:]
    dtA_t = nc.alloc_sbuf_tensor("dtA", [128, GRP * NG], fp8)
    dtB_t = nc.alloc_sbuf_tensor("dtB", [128, 2 * GRP * NG], fp8)
    dtA, dtB = dtA_t.ap(), dtB_t.ap()
    DTg = [dtA, dtB[:, 0:GRP * NG], dtB[:, GRP * NG:]]

    with tile.TileContext(nc) as tc:
        nc.scalar.dma_start(qwA[:], qw_d[:, 0:GRP * 128])
        nc.scalar.dma_start(qwB[:], qw_d[:, GRP * 128:])
        nc.sync.dma_start(dtA[:], dT_d[:, 0:GRP * NG])
        nc.sync.dma_start(dtB[:], dT_d[:, GRP * NG:])
        with (
            tc.tile_pool(name="small", bufs=1) as small,
            tc.tile_pool(name="ps", bufs=2, space="PSUM") as ps,
            tc.tile_pool(name="fin", bufs=1, space="PSUM") as fin,
        ):
            # A[p, parity, slot*16+doc] = relu(max over segment), bf16
            A = small.tile([128, 2, NSLOT * DSHARD], bf, tag="a")
            OUTS = small.tile([NQ, DSHARD], f32, tag="outs")
            pf = fin.tile([NQ, DSHARD], f32, tag="fin")
            ncnt = [0]

            def emit_finale(g):
                for k in range(2 * GRP):
                    par, sl = k % 2, g * GRP + k // 2
                    wb = (par * NSLOT + sl) * NQ
                    nc.tensor.matmul(pf[:], W2[:, wb:wb + NQ],
                                     A[:, par, sl * DSHARD:(sl + 1) * DSHARD],
                                     start=(ncnt[0] == 0),
                                     stop=(ncnt[0] == B - 1))
                    ncnt[0] += 1

            for g in range(NGRP):
                # one 2-bank PSUM tile per group: bank 0 = even buckets,
                # bank 1 = odd; each matmul stays within one bank
                ps_t = ps.tile([128, 2, 512], f32, tag="ps")
                for sl in range(GRP):
                    qe = QWg[g][0:64, sl * 128:(sl + 1) * 128]
                    qo = QWg[g][64:128, sl * 128:(sl + 1) * 128]
                    de = DTg[g][0:64, sl * NG:(sl + 1) * NG]
                    do = DTg[g][64:128, sl * NG:(sl + 1) * NG]
                    nc.tensor.matmul(ps_t[:, 0, sl * NG:(sl + 1) * NG],
                                     qe, de, start=True, stop=True)
                    nc.tensor.matmul(ps_t[:, 1, sl * NG:(sl + 1) * NG],
                                     qo, do, start=True, stop=True)
                # drain both banks with ONE segmented reduce straight from
                # PSUM into bf16 A (DVE; ScalarE/trees lose on overheads)
                nc.vector.reduce_max(
                    A[:, :, g * SEG:(g + 1) * SEG],
                    ps_t[:, :, 0:GRP * NG].rearrange("p a (s g) -> p a s g",
                                                     g=G),
                    axis=mybir.AxisListType.X)
            for g in range(NGRP):
                emit_finale(g)
            nc.vector.tensor_copy(OUTS[:], pf[:])
            nc.sync.dma_start(out_d[:], OUTS[:])

    nc.compile()
    return nc


def _get_nc():
    global _NC
    if _NC is None:
        _NC = _build_program()
    return _NC


def _install_ntff_shim():
    """Under axon the NTFF profile hook module may be missing; install it so
    trace=True returns exec_time_ns. Harmless no-op if already present."""
    import types
    try:
        import antenv.axon_hooks  # noqa: F401
        return
    except ImportError:
        pass
    try:
        from trn_agent_boot.trn_boot import _ntff_profile_via_ctypes
        hook = _ntff_profile_via_ctypes("/opt/axon/libaxon_pjrt.so")
        mod = types.ModuleType("antenv.axon_hooks")
        mod.get_axon_ntff_profile_hook = lambda: hook
        mod.set_axon_ntff_profile_hook = lambda h: None
        sys.modules["antenv.axon_hooks"] = mod
    except Exception:
        pass


def _run(in_maps, trace=False):
    from concourse.bass_utils import run_bass_kernel_spmd
    if trace:
        _install_ntff_shim()
    nc = _get_nc()
    res = run_bass_kernel_spmd(nc, in_maps, core_ids=list(range(NCORES)),
                               trace=trace)
    out = np.zeros((NQ, ND), dtype=np.float32)
    for core in range(NCORES):
        out[:, core * DSHARD:(core + 1) * DSHARD] = res.results[core]["out"]
    return out, res


def kernel(doc_reps, qry_reps, qry_attention_mask, doc_input_ids,
           qry_input_ids):
    in_maps = _prepare(doc_reps, qry_reps, qry_attention_mask,
                       doc_input_ids, qry_input_ids)
    out, _ = _run(in_maps, trace=False)
    return out


def kernel_traced(doc_reps, qry_reps, qry_attention_mask, doc_input_ids,
                  qry_input_ids):
    """Returns (output, exec_time_ns) using the NTFF profiling path."""
    in_maps = _prepare(doc_reps, qry_reps, qry_attention_mask,
                       doc_input_ids, qry_input_ids)
    out, res = _run(in_maps, trace=True)
    return out, res.exec_time_ns


# revision 22
# speedup vs baseline: 1.0727x; 1.0727x over previous
"""COIL-style retrieval scoring kernel for Trainium2 (8 NeuronCores, SPMD).

Problem: nn_BertForSemanticEmbedding_16973710754315
  out[q, n] = sum_{i>=1} mask[q,i] * max_j( where(qid[q,i]==did[n,j], qry[q,i]·doc[n,j], 0) )

Algorithm (docs sharded 16/core, queries replicated), "bucketed COIL":

  * Host partitions the 1000 vocab ids into B=18 buckets (greedy vector
    bin-packing + local repair) such that
      - each bucket holds <=128 query tokens  (matmul stationary M)
      - each (doc, bucket) token count <= 9   (so G=10 with a zero pad slot)
    Tokens can only exact-match within their id's bucket, so each device
    scores 18 bucket-local matmuls [K=64, M=128] x [K=64, N=160] instead of
    a dense 2048x2048 sweep -- ~11x less post-matmul reduce volume.
  * Exact-match discrimination INSIDE the matmul: each token's 63-dim
    augmented vector is [reps(32) | code(id)(30) | bias(1)] with codes +-4
    and bias q:-30 / d:16 (both e4m3-exact; product -480 = code self-dot).
    Matching ids contribute code.code - 480 = 0 exactly; in-bucket
    mismatches contribute <= 416 + |S| - 480 < 0 (the host verifies the
    in-bucket code-gram max <= 416, reseeding codes if needed).  Doc-side
    pad columns are all-zero, so every segment contains an exact 0 => the
    segmented max IS relu(max over matching S): no bias/relu op needed.
  * K=64 lets two buckets run CONCURRENTLY in the PE array via row tiling:
    even bucket in array rows 0-63, odd in rows 64-127 (tile_position is
    auto-derived from the operands' base_partition).  9 slot pairs.
  * dT ships as fp8e4m3 (codes/bias exact, reps ~6e-3 extra rel err;
    mixed bf16 x fp8 matmul works); qw/w2 stay bf16 -- fp8's narrow
    per-partition DMA lines measured slower despite fewer bytes.
  * Inputs are chunked per slot-group on two HWDGE queues (qw on the
    Activation queue, dT + w2 on SP), issued before the tile pools so
    descriptor generation overlaps the framework preamble; the matmul
    stream starts as soon as chunk 0 lands.
  * PSUM: one 2-bank tile per group holds 3 even + 3 odd buckets.  ONE
    DVE segmented reduce_max per group reads both banks straight from
    PSUM ([128, 2, 48, 10] -> bf16 A); ScalarE extract + tensor_max
    trees measured worse (per-op overhead + serial latency).
  * Finale: per bucket ONE accumulating matmul with the mask-scatter
    matrix W (qtok -> query, zero for [CLS]/pads) as stationary and the
    reduced A slice as moving operand; all 18 accumulate into one [16,16]
    PSUM tile.  VectorE copies it out (no ScalarE anywhere => no ACT
    table load blocking the DGE queue); DMA [16,16] f32 per core.

  Measured: 47071 ns (dense-COIL baseline) -> 18576 ns.  Of the total,
  ~13 us is fixed framework preamble + semaphore-sweep epilogue that any
  kernel on this harness pays; the compute body is ~5.5 us, paced by
  input-DMA arrival and the serial DVE reduce chain.
"""

import sys
import numpy as np

for _p in ("/opt/trn_rl_repo",):
    if _p not in sys.path:
        sys.path.insert(0, _p)

import ml_dtypes

BF16 = ml_dtypes.bfloat16

NQ, LQ = 16, 128
ND, LD = 128, 128
D = 32
VOCAB = 1000
NCORES = 8
DSHARD = ND // NCORES   # 16 docs per core
NQTOK = NQ * LQ         # 2048 query tokens

R = 31                  # code dims
CVAL = 4.0              # code magnitude (exact in bf16)
BIAS = float(R * CVAL * CVAL)  # 496 = code self-dot, cancelled by bias dim
GRAM_MAX = 448.0        # forbid in-bucket cross-grams >= this (=> <= 432)
KAUG = D + R + 1        # 64 = contraction dim; 2 buckets pack in the PE
B = 18                  # id buckets
CAP = 9                 # max doc tokens per (doc, bucket)
G = CAP + 1             # segment size incl >=1 zero pad slot
NG = DSHARD * G         # 160 = matmul N per bucket
NSLOT = B // 2          # 9 row-tiled matmul pairs
GRP = 3                 # slots per PSUM bank group
NGRP = NSLOT // GRP     # 3 slot groups
SEG = GRP * DSHARD      # 48 segments per bank
E4M3 = (ml_dtypes.float8_e4m3fn if hasattr(ml_dtypes, "float8_e4m3fn")
        else ml_dtypes.float8_e4m3)

_NC = None


# ---------------------------------------------------------------- host prep

def _pack_buckets(qc, dc):
    """Greedy vector bin-packing of ids into B buckets + local repair.
    qc: [VOCAB] query-token counts; dc: [VOCAB, ND] doc-token counts.
    Returns assign [VOCAB] with per-bucket qload<=128 and cell<=CAP."""
    QCAP = 128
    for seed in range(16):
        rng = np.random.RandomState(seed)
        noise = rng.rand(VOCAB) * 0.5
        order = np.argsort(-(dc.max(axis=1) * 100 + dc.sum(axis=1) + qc + noise))
        assign = np.full(VOCAB, -1, dtype=np.int64)
        cell = np.zeros((B, ND), dtype=np.int64)
        qload = np.zeros(B, dtype=np.int64)
        for v in order:
            nc_ = cell + dc[v][None, :]
            over = np.maximum(nc_ - CAP, 0).sum(axis=1)
            qbad = (qload + qc[v]) > QCAP
            score = (over * 10000 + qbad * 10**8
                     + cell.sum(axis=1) + qload * 2 + rng.rand(B))
            b = int(np.argmin(score))
            assign[v] = b
            cell[b] += dc[v]
            qload[b] += qc[v]

        def violations():
            return int(np.maximum(cell - CAP, 0).sum()
                       + np.maximum(qload - QCAP, 0).sum())

        vi = violations()
        for _ in range(20000):
            if vi == 0:
                break
            ob, od = np.nonzero(cell > CAP)
            if len(ob) == 0:
                oq = np.nonzero(qload > QCAP)[0]
                b0, d0 = int(oq[rng.randint(len(oq))]), None
            else:
                j = rng.randint(len(ob))
                b0, d0 = int(ob[j]), int(od[j])
            cand = np.nonzero((assign == b0) & ((dc[:, d0] > 0) if d0 is not None
                                                else (qc > 0)))[0]
            if len(cand) == 0:
                continue
            v = int(cand[rng.randint(len(cand))])
            nc_ = cell + dc[v][None, :]
            over_add = (np.maximum(nc_ - CAP, 0).sum(axis=1)
                        - np.maximum(cell - CAP, 0).sum(axis=1))
            q_add = (np.maximum(qload + qc[v] - QCAP, 0)
                     - np.maximum(qload - QCAP, 0))
            over_rem = (np.maximum(cell[b0] - CAP, 0).sum()
                        - np.maximum(cell[b0] - dc[v] - CAP, 0).sum())
            q_rem = (max(qload[b0] - QCAP, 0)
                     - max(qload[b0] - qc[v] - QCAP, 0))
            delta = over_add + q_add - over_rem - q_rem
            delta[b0] = 10**9
            b1 = int(np.argmin(delta + rng.rand(B) * 0.01))
            if delta[b1] < 0 or (delta[b1] == 0 and rng.rand() < 0.3):
                assign[v] = b1
                cell[b0] -= dc[v]
                cell[b1] += dc[v]
                qload[b0] -= qc[v]
                qload[b1] += qc[v]
                vi = violations()
        if vi == 0:
            return assign
    raise RuntimeError("bucket packing failed")


def _make_codes(assign, q_present, d_present):
    """[VOCAB, R] codes +-CVAL whose in-bucket co-occurring cross-grams
    stay < GRAM_MAX (so mismatch scores are strictly negative)."""
    for seed in range(64):
        rng = np.random.RandomState(12345 + seed)
        C = np.where(rng.rand(VOCAB, R) < 0.5, -CVAL, CVAL).astype(np.float32)
        gram = C @ C.T
        bad = False
        for b in range(B):
            ids = np.nonzero(assign == b)[0]
            qi = ids[q_present[ids]]
            di = ids[d_present[ids]]
            if len(qi) == 0 or len(di) == 0:
                continue
            g = gram[np.ix_(qi, di)].copy()
            g[qi[:, None] == di[None, :]] = -1e9
            if g.max() >= GRAM_MAX:
                bad = True
                break
        if not bad:
            return C
    raise RuntimeError("code generation failed")


def _prepare(doc_reps, qry_reps, qry_attention_mask, doc_input_ids,
             qry_input_ids):
    """Returns per-core input maps: bucketed, padded, bf16 device layouts."""
    qry_reps = np.asarray(qry_reps, dtype=np.float32).reshape(NQTOK, D)
    doc_reps = np.asarray(doc_reps, dtype=np.float32).reshape(ND * LD, D)
    mask = np.asarray(qry_attention_mask, dtype=np.float32)
    qids = np.asarray(qry_input_ids).astype(np.int64).reshape(NQTOK)
    dids = np.asarray(doc_input_ids).astype(np.int64).reshape(ND, LD)

    qc = np.bincount(qids, minlength=VOCAB)
    dc = np.zeros((VOCAB, ND), dtype=np.int64)
    for n in range(ND):
        dc[:, n] += np.bincount(dids[n], minlength=VOCAB)

    assign = _pack_buckets(qc, dc)
    C = _make_codes(assign, qc > 0, dc.sum(axis=1) > 0)

    # augmented token vectors [*, 64]
    qaug = np.zeros((NQTOK, KAUG), dtype=np.float32)
    qaug[:, :D] = qry_reps
    qaug[:, D:D + R] = C[qids]
    qaug[:, D + R] = -BIAS
    daug = np.zeros((ND * LD, KAUG), dtype=np.float32)
    daug[:, :D] = doc_reps
    daug[:, D:D + R] = C[dids.reshape(-1)]
    daug[:, D + R] = 1.0

    tok_bucket = assign[qids]
    W = mask.copy()
    W[:, 0] = 0.0                                   # skip [CLS]
    # qT [128, NSLOT*128]: slot s cols; even-bucket dims on partitions 0:64,
    # odd on 64:128.  w2 [128, 2*NSLOT*NQ] flat [parity][slot][query].
    qT = np.zeros((128, NSLOT * 128), dtype=np.float32)
    w2 = np.zeros((128, 2, NSLOT, NQ), dtype=np.float32)
    for b in range(B):
        s, par = divmod(b, 2)
        toks = np.nonzero(tok_bucket == b)[0]
        assert len(toks) <= 128, f"bucket {b} has {len(toks)} query tokens"
        prow = slice(0, KAUG) if par == 0 else slice(64, 64 + KAUG)
        qT[prow, s * 128:s * 128 + len(toks)] = qaug[toks].T
        qq, ii = toks // LQ, toks % LQ
        w2[np.arange(len(toks)), par, s, qq] = W[qq, ii]

    # doc-side per core: dT [128, NSLOT*NG]; slot s cols are
    # [doc0: G slots | ... | doc15: G slots], zero padded.
    d_bucket = assign[dids]
    qw_bf = qT.astype(BF16)
    w2_bf = np.ascontiguousarray(w2.reshape(128, 2 * NSLOT * NQ)).astype(BF16)
    in_maps = []
    for core in range(NCORES):
        dT = np.zeros((128, NSLOT * NG), dtype=np.float32)
        for nl in range(DSHARD):
            n = core * DSHARD + nl
            for b in range(B):
                s, par = divmod(b, 2)
                js = np.nonzero(d_bucket[n] == b)[0]
                assert len(js) <= CAP, f"doc {n} bucket {b}: {len(js)}"
                col = s * NG + nl * G
                prow = slice(0, KAUG) if par == 0 else slice(64, 64 + KAUG)
                dT[prow, col:col + len(js)] = daug[n * LD + js].T
        in_maps.append({"qw": qw_bf, "dT": dT.astype(E4M3), "w2": w2_bf})
    return in_maps


# ---------------------------------------------------------------- program

def _build_program():
    from concourse import bacc, tile, mybir

    bf = mybir.dt.bfloat16
    f32 = mybir.dt.float32

    nc = bacc.Bacc("TRN2", target_bir_lowering=False, debug=False,
                   num_devices=NCORES)

    fp8 = mybir.dt.float8e4
    qw_d = nc.declare_dram_parameter("qw", [128, NSLOT * 128], bf,
                                     isOutput=False)
    dT_d = nc.declare_dram_parameter("dT", [128, NSLOT * NG], fp8,
                                     isOutput=False)
    w2_d = nc.declare_dram_parameter("w2", [128, 2 * NSLOT * NQ], bf,
                                     isOutput=False)
    out_d = nc.declare_dram_parameter("out", [NQ, DSHARD], f32, isOutput=True)

    # Input tensors live outside the tile pools; their DMAs are the FIRST
    # instructions in the TileContext, ahead of the tile-pool allocation
    # barriers, so descriptor generation starts as soon as the engine
    # queues open.  Inputs are split per slot-group so group-0 matmuls
    # start early; qw chunks ride the Activation DGE queue, dT chunks +
    # w2 the SP queue.
    QWg, DTg = [], []
    for g in range(NGRP):
        qt_t = nc.alloc_sbuf_tensor(f"qw{g}", [128, GRP * 128], bf)
        dt_t = nc.alloc_sbuf_tensor(f"dt{g}", [128, GRP * NG], fp8)
        QWg.append(qt_t.ap())
        DTg.append(dt_t.ap())
    w2_t = nc.alloc_sbuf_tensor("w2t", [128, 2 * NSLOT * NQ], bf)
    W2 = w2_t.ap()

    with tile.TileContext(nc) as tc:
        for g in range(NGRP):
            lo = g * GRP * 128
            nc.scalar.dma_start(QWg[g][:], qw_d[:, lo:lo + GRP * 128])
            nc.sync.dma_start(DTg[g][:], dT_d[:, g * GRP * NG:(g + 1) * GRP * NG])
        nc.sync.dma_start(W2[:], w2_d[:])
        with (
            tc.tile_pool(name="small", bufs=1) as small,
            tc.tile_pool(name="ps", bufs=2, space="PSUM") as ps,
            tc.tile_pool(name="fin", bufs=1, space="PSUM") as fin,
        ):
            # A[p, parity, slot*16+doc] = relu(max over segment), bf16
            A = small.tile([128, 2, NSLOT * DSHARD], bf, tag="a")
            OUTS = small.tile([NQ, DSHARD], f32, tag="outs")
            pf = fin.tile([NQ, DSHARD], f32, tag="fin")
            ncnt = [0]

            def emit_finale(g):
                for k in range(2 * GRP):
                    par, sl = k % 2, g * GRP + k // 2
                    wb = (par * NSLOT + sl) * NQ
                    nc.tensor.matmul(pf[:], W2[:, wb:wb + NQ],
                                     A[:, par, sl * DSHARD:(sl + 1) * DSHARD],
                                     start=(ncnt[0] == 0),
                                     stop=(ncnt[0] == B - 1))
                    ncnt[0] += 1

            for g in range(NGRP):
                # one 2-bank PSUM tile per group: bank 0 = even buckets,
                # bank 1 = odd; each matmul stays within one bank
                ps_t = ps.tile([128, 2, 512], f32, tag="ps")
                for sl in range(GRP):
                    qe = QWg[g][0:64, sl * 128:(sl + 1) * 128]
                    qo = QWg[g][64:128, sl * 128:(sl + 1) * 128]
                    de = DTg[g][0:64, sl * NG:(sl + 1) * NG]
                    do = DTg[g][64:128, sl * NG:(sl + 1) * NG]
                    nc.tensor.matmul(ps_t[:, 0, sl * NG:(sl + 1) * NG],
                                     qe, de, start=True, stop=True)
                    nc.tensor.matmul(ps_t[:, 1, sl * NG:(sl + 1) * NG],
                                     qo, do, start=True, stop=True)
                # drain both banks with ONE segmented reduce straight from
                # PSUM into bf16 A (DVE; ScalarE/trees lose on overheads)
                nc.vector.reduce_max(
                    A[:, :, g * SEG:(g + 1) * SEG],
                    ps_t[:, :, 0:GRP * NG].rearrange("p a (s g) -> p a s g",
                                                     g=G),
                    axis=mybir.AxisListType.X)
            for g in range(NGRP):
                emit_finale(g)
            nc.vector.tensor_copy(OUTS[:], pf[:])
            nc.sync.dma_start(out_d[:], OUTS[:])

    nc.compile()
    return nc


def _get_nc():
    global _NC
    if _NC is None:
        _NC = _build_program()
    return _NC


def _install_ntff_shim():
    """Under axon the NTFF profile hook module may be missing; install it so
    trace=True returns exec_time_ns. Harmless no-op if already present."""
    import types
    try:
        import antenv.axon_hooks  # noqa: F401
        return
    except ImportError:
        pass
    try:
        from trn_agent_boot.trn_boot import _ntff_profile_via_ctypes
        hook = _ntff_profile_via_ctypes("/opt/axon/libaxon_pjrt.so")
        mod = types.ModuleType("antenv.axon_hooks")
        mod.get_axon_ntff_profile_hook = lambda: hook
        mod.set_axon_ntff_profile_hook = lambda h: None
        sys.modules["antenv.axon_hooks"] = mod
    except Exception:
        pass


def _run(in_maps, trace=False):
    from concourse.bass_utils import run_bass_kernel_spmd
    if trace:
        _install_ntff_shim()
    nc = _get_nc()
    res = run_bass_kernel_spmd(nc, in_maps, core_ids=list(range(NCORES)),
                               trace=trace)
    out = np.zeros((NQ, ND), dtype=np.float32)
    for core in range(NCORES):
        out[:, core * DSHARD:(core + 1) * DSHARD] = res.results[core]["out"]
    return out, res


def kernel(doc_reps, qry_reps, qry_attention_mask, doc_input_ids,
           qry_input_ids):
    in_maps = _prepare(doc_reps, qry_reps, qry_attention_mask,
                       doc_input_ids, qry_input_ids)
    out, _ = _run(in_maps, trace=False)
    return out


def kernel_traced(doc_reps, qry_reps, qry_attention_mask, doc_input_ids,
                  qry_input_ids):
    """Returns (output, exec_time_ns) using the NTFF profiling path."""
    in_maps = _prepare(doc_reps, qry_reps, qry_attention_mask,
                       doc_input_ids, qry_input_ids)
    out, res = _run(in_maps, trace=True)
    return out, res.exec_time_ns
